# revision 1
# baseline (speedup 1.0000x reference)
"""Trainium2 Bass kernel for the grouped contrastive loss.

Math: for anchors i and positives j restricted to the same
sensitive-attribute group g (size P),
    row(i,j) = S_ij - D * log E_ij
with S_ij = <p_i, p_j>/t and E_ij = sum_d exp(p_i[d] p_j[d] / t)
(the log-softmax max-shift cancels analytically), and
    loss = sum_i -1/(N P_i^2) * sum_{j in g(i)} row(i,j).

Strategy: sort points by group host-side so the same-group mask becomes
dense per-group blocks. Work = slots, each slot = (block of <=128 sorted
anchors, j-window of <=W columns of that block's group). Per slot, on
device (anchors on partitions as 32 packs of 4 anchors x 32 dims):
  - S via one fp32 matmul (lhsT = anchor points [32,128], rhs = window
    points [32,W]).
  - E via: DVE tensor_scalar broadcast-multiply (per-pack scalar column
    against 4x-replicated window points), ACT exp (batched 8 packs), and
    per-pack bf16 matmuls against shifted block-diagonal ones that
    accumulate the 32 exp rows of each anchor into its PSUM row.
  - Ln on ACT with accum_out gives sum_j log E per anchor for free.
Dummy rows/columns are weighted out host-side (w=0) or corrected by the
exact constant D*ln(D)*n_dummy per slot. The 8 cores run one SPMD
program over per-core input arrays; each returns a [128] partial that the
host sums.
"""

import math
import os
import sys

sys.path.insert(0, "/opt/trn_rl_repo")

import numpy as np
import ml_dtypes

import concourse.bacc as bacc
import concourse.bass as bass
import concourse.tile as tile
from concourse import mybir
from concourse.bass_utils import run_bass_kernel_spmd

N_CORES = 8
D = 32
PACKS = 32  # packs of 4 anchors per 128-anchor block

last_run_info = {}


def _install_drain_split_patch():
    # This walrus build rejects Drain instructions carrying more than one
    # semaphore wait ("Too many sync wait commands"). TileContext's exit
    # emits one kernel-tail Drain with a wait per outstanding logical
    # processor; split the extras across additional single-wait Drains on
    # the same engine (sequential waits are semantically identical).
    import concourse.tile as tile_mod

    if getattr(tile_mod.TileContext, "_drain_split_patched", False):
        return

    def _drain_and_barrier(self, tick_clock, wait_clock):
        nc = self.nc
        drain_inst = nc.sync.drain()
        wait_clock.add_sem_waits(
            drain_inst.ins,
            tile_mod.ScopedClock({None: tick_clock.global_clock}),
        )
        si = drain_inst.ins.sync_info
        if si is not None and si.on_wait is not None and len(si.on_wait) > 1:
            waits = list(si.on_wait)
            si.on_wait = [waits[0]]
            for w in waits[1:]:
                d2 = nc.sync.drain()
                si2 = d2.ins.sync_info
                if si2 is None:
                    d2.ins.sync_info = type(si)(on_wait=[w], on_update=[])
                else:
                    si2.on_wait = [w]

        nc.all_engine_barrier()
        assert self.sems is not None
        popped = nc._tile_sem_poison_stack.pop()
        assert popped is self._sem_poison
        nc.clear_and_free_semaphores(list(self.sems.allocated().values()))
        nc.all_engine_barrier()

    tile_mod.TileContext._drain_and_barrier = _drain_and_barrier
    tile_mod.TileContext._drain_split_patched = True


def _install_ntff_hook():
    # bass_utils' trace path under axon imports antenv.axon_hooks, which is
    # absent in this image; provide the ctypes-based hook it expects.
    import contextlib
    import ctypes
    import types

    if "antenv.axon_hooks" in sys.modules:
        return

    def _make_hook():
        try:
            lib = ctypes.CDLL("/opt/axon/libaxon_pjrt.so")
        except OSError:
            return None
        if not hasattr(lib, "axon_start_nrt_profile"):
            return None
        lib.axon_start_nrt_profile.argtypes = [
            ctypes.POINTER(ctypes.c_int64),
            ctypes.c_size_t,
        ]
        lib.axon_start_nrt_profile.restype = ctypes.c_int64
        lib.axon_stop_nrt_profile.argtypes = [ctypes.c_char_p]
        lib.axon_stop_nrt_profile.restype = ctypes.c_int64

        @contextlib.contextmanager
        def _hook_cm(output_dir, device_ids):
            import jax

            jax.devices()
            if device_ids:
                ids = (ctypes.c_int64 * len(device_ids))(*device_ids)
                rc = lib.axon_start_nrt_profile(ids, len(device_ids))
            else:
                rc = lib.axon_start_nrt_profile(None, 0)
            if rc != 0:
                raise RuntimeError(f"axon_start_nrt_profile rc={rc}")
            try:
                yield
            finally:
                n = lib.axon_stop_nrt_profile(str(output_dir).encode())
                if n < 0:
                    raise RuntimeError(f"axon_stop_nrt_profile rc={n}")

        return _hook_cm

    hook = _make_hook()
    mod = types.ModuleType("antenv.axon_hooks")
    mod.get_axon_ntff_profile_hook = lambda: hook
    mod.set_axon_ntff_profile_hook = lambda h: None
    sys.modules["antenv.axon_hooks"] = mod


def _plan(sa_sorted):
    """Slot plan from the sorted attribute vector.

    Returns (W, ntiles, slots_per_core) where each slot is
    (pos0, row_lo, row_hi, g0, g1, c0, L):
      pos0: first sorted-anchor position of the 128-anchor block
      [row_lo, row_hi): rows of the block belonging to group [g0, g1)
      [c0, c0+L): this slot's j-window (sorted positions) within the group
    or None for a dummy slot.
    """
    n = len(sa_sorted)
    assert n % 128 == 0
    n_blocks = n // 128
    bounds = [0]
    for i in range(1, n):
        if sa_sorted[i] != sa_sorted[i - 1]:
            bounds.append(i)
    bounds.append(n)

    jobs = []  # (pos0, row_lo, row_hi, g0, g1)
    for b in range(n_blocks):
        pos0 = b * 128
        for gi in range(len(bounds) - 1):
            g0, g1 = bounds[gi], bounds[gi + 1]
            lo = max(pos0, g0)
            hi = min(pos0 + 128, g1)
            if lo < hi:
                jobs.append((pos0, lo - pos0, hi - pos0, g0, g1))

    best = None
    for W in range(128, 513, 16):
        T = sum((g1 - g0 + W - 1) // W for (_, _, _, g0, g1) in jobs)
        ntiles = (T + N_CORES - 1) // N_CORES
        cost = ntiles * W
        if best is None or cost < best[0] or (cost == best[0] and W > best[1]):
            best = (cost, W, ntiles)
    _, W, ntiles = best

    slots = []
    for pos0, row_lo, row_hi, g0, g1 in jobs:
        for c0 in range(g0, g1, W):
            L = min(W, g1 - c0)
            slots.append((pos0, row_lo, row_hi, g0, g1, c0, L))

    per_core = [[] for _ in range(N_CORES)]
    for i, s in enumerate(slots):
        per_core[i % N_CORES].append(s)
    for c in range(N_CORES):
        while len(per_core[c]) < ntiles:
            per_core[c].append(None)
    return W, ntiles, per_core


def _build_program(W, ntiles):
    # Bacc (not raw Bass): its compile() runs generate_event_semaphores,
    # which splits multi-semaphore waits to satisfy the TRN2 one-wait-per-
    # instruction constraint this walrus build enforces.
    nc = bacc.Bacc(
        "TRN2", target_bir_lowering=False, debug=False, num_devices=N_CORES
    )
    f32 = mybir.dt.float32
    bf16 = mybir.dt.bfloat16

    rep4_d = nc.dram_tensor("rep4", [128, ntiles * W], f32, kind="ExternalInput").ap()
    rhsj_d = nc.dram_tensor("rhsj", [32, ntiles * W], f32, kind="ExternalInput").ap()
    lhsa_d = nc.dram_tensor("lhsa", [32, ntiles * 128], f32, kind="ExternalInput").ap()
    scal_d = nc.dram_tensor("scal", [128, ntiles * PACKS], f32, kind="ExternalInput").ap()
    wcol_d = nc.dram_tensor("wcol", [128, ntiles], f32, kind="ExternalInput").ap()
    kcol_d = nc.dram_tensor("kcol", [128, ntiles], f32, kind="ExternalInput").ap()
    ones_d = nc.dram_tensor("onesbd", [128, 8 * 32], bf16, kind="ExternalInput").ap()
    out_d = nc.dram_tensor("out", [128, 1], f32, kind="ExternalOutput").ap()

    Exp = mybir.ActivationFunctionType.Exp
    Ln = mybir.ActivationFunctionType.Ln

    with tile.TileContext(nc) as tc:
        with (
            tc.tile_pool(name="const", bufs=1) as cpool,
            tc.tile_pool(name="work", bufs=3) as wpool,
            tc.tile_pool(name="red", bufs=2) as rpool,
            tc.tile_pool(name="psE", bufs=2, space="PSUM") as psE,
            tc.tile_pool(name="psS", bufs=2, space="PSUM") as psS,
            tc.tile_pool(name="psL", bufs=1, space="PSUM") as psL,
        ):
            rep4 = cpool.tile([128, ntiles * W], f32, tag="rep4")
            nc.gpsimd.dma_start(rep4[:], rep4_d[:])
            rhsj = cpool.tile([32, ntiles * W], f32, tag="rhsj")
            nc.gpsimd.dma_start(rhsj[:], rhsj_d[:])
            lhsa = cpool.tile([32, ntiles * 128], f32, tag="lhsa")
            nc.gpsimd.dma_start(lhsa[:], lhsa_d[:])
            scal = cpool.tile([128, ntiles * PACKS], f32, tag="scal")
            nc.gpsimd.dma_start(scal[:], scal_d[:])
            wcol = cpool.tile([128, ntiles], f32, tag="wcol")
            nc.gpsimd.dma_start(wcol[:], wcol_d[:])
            kcol = cpool.tile([128, ntiles], f32, tag="kcol")
            nc.gpsimd.dma_start(kcol[:], kcol_d[:])
            onesbd = cpool.tile([128, 8 * 32], bf16, tag="onesbd")
            nc.gpsimd.dma_start(onesbd[:], ones_d[:])

            acc = cpool.tile([128, 1], f32, tag="acc")
            nc.vector.memset(acc[:], 0.0)

            for s in range(ntiles):
                S_ps = psS.tile([128, W], f32, tag="S")
                nc.tensor.matmul(
                    S_ps[:],
                    lhsT=lhsa[:, s * 128 : (s + 1) * 128],
                    rhs=rhsj[:, s * W : (s + 1) * W],
                    start=True,
                    stop=True,
                )
                # PSUM APs can only start at partition 0/32/64, so the 128
                # anchor rows of E live in two [64, W] tiles.
                E_lo = psE.tile([64, W], f32, tag="Elo")
                E_hi = psE.tile([64, W], f32, tag="Ehi")
                for h in range(4):
                    prod = wpool.tile([128, 8 * W], f32, tag="prod")
                    for i in range(8):
                        k = 8 * h + i
                        nc.vector.tensor_scalar_mul(
                            prod[:, i * W : (i + 1) * W],
                            rep4[:, s * W : (s + 1) * W],
                            scal[:, s * PACKS + k : s * PACKS + k + 1],
                        )
                    expt = wpool.tile([128, 8 * W], bf16, tag="expt")
                    nc.scalar.activation(expt[:], prod[:], Exp)
                    E_t = E_lo if h < 2 else E_hi
                    rb = 32 * (h % 2)
                    for i in range(8):
                        nc.tensor.matmul(
                            E_t[rb : rb + 32, :],
                            lhsT=onesbd[:, 32 * i : 32 * (i + 1)],
                            rhs=expt[:, i * W : (i + 1) * W],
                            start=(i == 0),
                            stop=(i == 7),
                        )
                logE = psL.tile([128, W], f32, tag="logE")
                sL = rpool.tile([128, 1], f32, tag="sL")
                nc.scalar.activation(logE[0:64, :], E_lo[:], Ln, accum_out=sL[0:64, :])
                nc.scalar.activation(logE[64:128, :], E_hi[:], Ln, accum_out=sL[64:128, :])
                sS = rpool.tile([128, 1], f32, tag="sS")
                nc.vector.tensor_reduce(
                    sS[:], S_ps[:], axis=mybir.AxisListType.X, op=mybir.AluOpType.add
                )
                v1 = rpool.tile([128, 1], f32, tag="v1")
                nc.vector.tensor_scalar(
                    v1[:],
                    sL[:],
                    -float(D),
                    kcol[:, s : s + 1],
                    op0=mybir.AluOpType.mult,
                    op1=mybir.AluOpType.add,
                )
                v2 = rpool.tile([128, 1], f32, tag="v2")
                nc.vector.tensor_add(v2[:], v1[:], sS[:])
                nc.vector.scalar_tensor_tensor(
                    acc[:],
                    v2[:],
                    wcol[:, s : s + 1],
                    acc[:],
                    op0=mybir.AluOpType.mult,
                    op1=mybir.AluOpType.add,
                )

            nc.gpsimd.dma_start(out_d[:], acc[:])

    nc.compile()
    return nc


def kernel(points, sensitive_attribute, t):
    _install_ntff_hook()

    points = np.asarray(points, dtype=np.float32)
    sa = np.asarray(sensitive_attribute).astype(np.int64)
    n, d = points.shape
    assert d == D

    scale = 1.0 / math.sqrt(float(np.asarray(t)))
    order = np.argsort(sa, kind="stable")
    sa_sorted = sa[order]
    ps = (points[order] * np.float32(scale)).astype(np.float32)  # [n, 32] sorted

    W, ntiles, per_core = _plan(sa_sorted)

    lnD = math.log(float(D))
    in_maps = []
    for c in range(N_CORES):
        rep4 = np.zeros((128, ntiles * W), np.float32)
        rhsj = np.zeros((32, ntiles * W), np.float32)
        lhsa = np.zeros((32, ntiles * 128), np.float32)
        scal = np.zeros((128, ntiles * PACKS), np.float32)
        wcol = np.zeros((128, ntiles), np.float32)
        kcol = np.zeros((128, ntiles), np.float32)
        for s, slot in enumerate(per_core[c]):
            if slot is None:
                # dummy slot: all-zero data; exp(0) rows sum to D, finite
                # log, zero weight. Correction value irrelevant (w=0).
                continue
            pos0, row_lo, row_hi, g0, g1, c0, L = slot
            P = g1 - g0
            win = ps[c0 : c0 + L].T  # [32, L]
            rhsj[:, s * W : s * W + L] = win
            rep4[:, s * W : s * W + L] = np.tile(win, (4, 1))
            ablk = np.zeros((32, 128), np.float32)
            ablk[:, row_lo:row_hi] = ps[pos0 + row_lo : pos0 + row_hi].T
            lhsa[:, s * 128 : (s + 1) * 128] = ablk
            # scal column k = anchors 4k..4k+3 flattened (a-major, d-minor)
            scal[:, s * PACKS : (s + 1) * PACKS] = (
                ablk.T.reshape(PACKS, 128).T
            )
            wcol[row_lo:row_hi, s] = -1.0 / (n * float(P) * float(P))
            kcol[:, s] = D * lnD * (W - L)

        onesbd = np.zeros((128, 8 * 32), ml_dtypes.bfloat16)
        for r in range(8):
            for a in range(4):
                onesbd[32 * a : 32 * (a + 1), 32 * r + 4 * r + a] = 1.0
        in_maps.append(
            {
                "rep4": rep4,
                "rhsj": rhsj,
                "lhsa": lhsa,
                "scal": scal,
                "wcol": wcol,
                "kcol": kcol,
                "onesbd": onesbd,
            }
        )

    nc = _build_program(W, ntiles)
    trace = bool(int(os.environ.get("KERNEL_TRACE", "0")))
    res = run_bass_kernel_spmd(nc, in_maps, list(range(N_CORES)), trace=trace)
    last_run_info["exec_time_ns"] = res.exec_time_ns
    last_run_info["mean_exec_time_ns"] = res.mean_exec_time_ns
    last_run_info["W"] = W
    last_run_info["ntiles"] = ntiles
    last_run_info["instructions"] = (
        res.instructions_and_trace[0] if res.instructions_and_trace else None
    )

    total = 0.0
    for c in range(N_CORES):
        total += float(res.results[c]["out"].astype(np.float64).sum())
    return np.float32(total)



# revision 8
# speedup vs baseline: 1.9699x; 1.9699x over previous
"""Trainium2 Bass kernel for the grouped contrastive loss (v2).

Math: for anchors i and positives j in the same sensitive-attribute
group g (size P),
    row(i,j) = S_ij - D * ln E_ij
with S_ij = <p_i, p_j>/t and E_ij = sum_d exp(p_i[d] p_j[d] / t)
(the log-softmax max-shift cancels analytically), and
    loss = sum_i -1/(N P_i^2) * sum_{j in g(i)} row(i,j).

v2 exploits the symmetry row(i,j) == row(j,i): sort points by group so
same-group pairs are dense blocks; for each (128-anchor block, group)
job the device computes one window = [own-block cols (padded to 128) |
suffix = all later same-group cols].  Within-block ordered pairs are
counted at weight 1 (both orders present in the own part); cross-block
unordered pairs appear once in the earlier block's suffix and get
weight 2.  Per slot the device produces
    Ssum[i]  = x_i . ybar_m          (ybar_m = sum_own y + 2*sum_sfx y)
    Lred[i]  = sum_{j in window} ln E_ij
    Lown[i]  = sum_{j in first 128 cols} ln E_ij
and the final per-row combine is
    acc = sum_s wS*Ssum + wA*Lred + wB*Lown + kwcol
with wA = -2*D*wS, wB = +D*wS (own part => net weight 1), and kwcol the
exact host-side correction for zero-padded dummy columns (E = D there).

Device pipeline per slot (W columns, 128 anchors as 32 packs x 4):
  DVE  : 32x tensor_scalar_mul bf16 -> prod [128, 32W]
  ACT  : 4x Exp chunks [128, 8W] (bf16), 1x Ln [128, W] PSUM->SBUF
         (Exp+Ln forced into the one 'natural_log_exp_and_others'
         table set -- no per-slot ACT_TABLE_LOADs)
  PE   : 32x block-diag matmuls accumulate exp rows -> E [128, W] PSUM,
         plus one 1-column fp32 matmul for Ssum
  DVE  : 2x tensor_reduce of ln E -> Lred/Lown columns
Final: 3 chained tensor_tensor_reduce -> acc [128,1]; host sums cores.
"""

import math
import os
import sys

sys.path.insert(0, "/opt/trn_rl_repo")

import numpy as np
import ml_dtypes

import concourse.bacc as bacc
import concourse.bass as bass
import concourse.tile as tile
from concourse import mybir
from concourse.bass_utils import run_bass_kernel_spmd

N_CORES = 8
D = 32
LN_D = math.log(float(D))
SPLIT = 384  # max window width (PSUM bank limit 512 fp32; 384 packs best)

last_run_info = {}


def _install_ntff_hook():
    # bass_utils' trace path under axon imports antenv.axon_hooks, which is
    # absent in this image; provide the ctypes-based hook it expects.
    import contextlib
    import ctypes
    import types

    if "antenv.axon_hooks" in sys.modules:
        return

    def _make_hook():
        try:
            lib = ctypes.CDLL("/opt/axon/libaxon_pjrt.so")
        except OSError:
            return None
        if not hasattr(lib, "axon_start_nrt_profile"):
            return None
        lib.axon_start_nrt_profile.argtypes = [
            ctypes.POINTER(ctypes.c_int64),
            ctypes.c_size_t,
        ]
        lib.axon_start_nrt_profile.restype = ctypes.c_int64
        lib.axon_stop_nrt_profile.argtypes = [ctypes.c_char_p]
        lib.axon_stop_nrt_profile.restype = ctypes.c_int64

        @contextlib.contextmanager
        def _hook_cm(output_dir, device_ids):
            import jax

            jax.devices()
            if device_ids:
                ids = (ctypes.c_int64 * len(device_ids))(*device_ids)
                rc = lib.axon_start_nrt_profile(ids, len(device_ids))
            else:
                rc = lib.axon_start_nrt_profile(None, 0)
            if rc != 0:
                raise RuntimeError(f"axon_start_nrt_profile rc={rc}")
            try:
                yield
            finally:
                n = lib.axon_stop_nrt_profile(str(output_dir).encode())
                if n < 0:
                    raise RuntimeError(f"axon_stop_nrt_profile rc={n}")

        return _hook_cm

    hook = _make_hook()
    mod = types.ModuleType("antenv.axon_hooks")
    mod.get_axon_ntff_profile_hook = lambda: hook
    mod.set_axon_ntff_profile_hook = lambda h: None
    sys.modules["antenv.axon_hooks"] = mod


def _install_act_table_patch():
    # Exp and Ln both live in the 'natural_log_exp_and_others' activation
    # table set; by default the table-load inserter resolves each function
    # to the first set containing it, so alternating Exp/Ln reloads tables
    # every transition (~1.3us each).  Strip Exp/Ln from every other set so
    # both resolve to the combined one -> a single hoisted load.
    if os.environ.get("ACT_TBL_PATCH", "1") != "1":
        return
    if getattr(bacc, "_act_tbl_patched", False):
        return
    orig = bacc.get_activation_tables
    Exp = mybir.ActivationFunctionType.Exp
    Ln = mybir.ActivationFunctionType.Ln

    def patched(arch):
        tabs = orig(arch)
        combined = "natural_log_exp_and_others"
        if combined not in tabs or not {Exp, Ln} <= tabs[combined]:
            return tabs
        return {
            name: (fns if name == combined else fns - {Exp, Ln})
            for name, fns in tabs.items()
        }

    bacc.get_activation_tables = patched
    bacc._act_tbl_patched = True


def _plan(sa_sorted):
    """Build symmetric slots and assign to cores.

    Slot = dict(pos0, lo, hi, P, own=(col0, L1) | None, sfx=(col0, ls), w).
    Rows of the slot are sorted positions [pos0+lo, pos0+hi); window
    layout: own cols at local [0, L1), zeros to 128, suffix at [128, ...)
    for own-slots; pure suffix at [0, ls) for tail slots.

    Returns (widths, per_core) with per_core[c] a list of len(widths)
    entries (slot dict or None), widths[s] the compile-time window width.
    """
    n = len(sa_sorted)
    assert n % 128 == 0
    bounds = [0]
    for i in range(1, n):
        if sa_sorted[i] != sa_sorted[i - 1]:
            bounds.append(i)
    bounds.append(n)
    groups = [(bounds[i], bounds[i + 1]) for i in range(len(bounds) - 1)]

    slots = []
    for b in range(n // 128):
        pos0 = 128 * b
        for g0, g1 in groups:
            lo, hi = max(pos0, g0), min(pos0 + 128, g1)
            if lo >= hi:
                continue
            P = g1 - g0
            L2 = max(0, g1 - (pos0 + 128))
            f = min(L2, SPLIT - 128)
            slots.append(
                dict(
                    pos0=pos0,
                    lo=lo - pos0,
                    hi=hi - pos0,
                    P=P,
                    own=(lo, hi - lo),
                    sfx=(pos0 + 128, f),
                    w=128 + f,
                )
            )
            c0, rem = pos0 + 128 + f, L2 - f
            while rem > 0:
                l = min(rem, SPLIT)
                slots.append(
                    dict(
                        pos0=pos0,
                        lo=lo - pos0,
                        hi=hi - pos0,
                        P=P,
                        own=None,
                        sfx=(c0, l),
                        w=l,
                    )
                )
                c0 += l
                rem -= l

    slots.sort(key=lambda s: -s["w"])
    ns = (len(slots) + N_CORES - 1) // N_CORES
    per_core = [[] for _ in range(N_CORES)]
    for k in range(ns):
        rank = slots[N_CORES * k : N_CORES * (k + 1)]
        order = range(N_CORES) if k % 2 == 0 else range(N_CORES - 1, -1, -1)
        it = iter(rank)
        assign = {}
        for c in order:
            assign[c] = next(it, None)
        for c in range(N_CORES):
            per_core[c].append(assign[c])
    widths = []
    for s in range(ns):
        wmax = max(p[s]["w"] if p[s] is not None else 0 for p in per_core)
        widths.append(max(32, int(math.ceil(wmax / 32.0)) * 32))
    return widths, per_core


def _build_program(widths):
    nc = bacc.Bacc(
        "TRN2", target_bir_lowering=False, debug=False, num_devices=N_CORES
    )
    f32 = mybir.dt.float32
    bf16 = mybir.dt.bfloat16
    ns = len(widths)
    TW = sum(widths)
    Wmax = max(widths)
    offs = [sum(widths[:s]) for s in range(ns)]

    rep4_d = nc.dram_tensor("rep4", [128, TW], bf16, kind="ExternalInput").ap()
    scal_d = nc.dram_tensor("scal", [128, ns * 32], f32, kind="ExternalInput").ap()
    lhsa_d = nc.dram_tensor("lhsa", [32, ns * 128], f32, kind="ExternalInput").ap()
    ysum_d = nc.dram_tensor("ysum", [32, ns], f32, kind="ExternalInput").ap()
    ws_d = nc.dram_tensor("ws", [128, ns], f32, kind="ExternalInput").ap()
    wa_d = nc.dram_tensor("wa", [128, ns], f32, kind="ExternalInput").ap()
    wb_d = nc.dram_tensor("wb", [128, ns], f32, kind="ExternalInput").ap()
    kw_d = nc.dram_tensor("kw", [128, 1], f32, kind="ExternalInput").ap()
    ones_d = nc.dram_tensor("onesbd", [128, 8 * 32], bf16, kind="ExternalInput").ap()
    out_d = nc.dram_tensor("out", [128, 1], f32, kind="ExternalOutput").ap()

    Exp = mybir.ActivationFunctionType.Exp
    Ln = mybir.ActivationFunctionType.Ln

    with tile.TileContext(nc) as tc:
        with (
            tc.tile_pool(name="const", bufs=1) as cpool,
            tc.tile_pool(name="prod", bufs=2) as ppool,
            tc.tile_pool(name="expt", bufs=2) as epool,
            tc.tile_pool(name="red", bufs=1) as rpool,
            tc.tile_pool(name="psE", bufs=2, space="PSUM") as psE,
            tc.tile_pool(name="psS", bufs=1, space="PSUM") as psS,
        ):
            rep4 = cpool.tile([128, TW], bf16, tag="rep4")
            nc.gpsimd.dma_start(rep4[:], rep4_d[:])
            scal = cpool.tile([128, ns * 32], f32, tag="scal")
            nc.gpsimd.dma_start(scal[:], scal_d[:])
            lhsa = cpool.tile([32, ns * 128], f32, tag="lhsa")
            nc.gpsimd.dma_start(lhsa[:], lhsa_d[:])
            ysum = cpool.tile([32, ns], f32, tag="ysum")
            nc.gpsimd.dma_start(ysum[:], ysum_d[:])
            wS = cpool.tile([128, ns], f32, tag="ws")
            nc.gpsimd.dma_start(wS[:], ws_d[:])
            wA = cpool.tile([128, ns], f32, tag="wa")
            nc.gpsimd.dma_start(wA[:], wa_d[:])
            wB = cpool.tile([128, ns], f32, tag="wb")
            nc.gpsimd.dma_start(wB[:], wb_d[:])
            kw = cpool.tile([128, 1], f32, tag="kw")
            nc.gpsimd.dma_start(kw[:], kw_d[:])
            onesbd = cpool.tile([128, 8 * 32], bf16, tag="onesbd")
            nc.gpsimd.dma_start(onesbd[:], ones_d[:])

            logE = cpool.tile([128, TW], f32, tag="logE")
            Lred = rpool.tile([128, ns], f32, tag="Lred")
            Lown = rpool.tile([128, ns], f32, tag="Lown")
            Ssum = psS.tile([128, ns], f32, tag="Ssum")

            for s in range(ns):
                W = widths[s]
                off = offs[s]
                prod = ppool.tile([128, 32 * Wmax], bf16, tag="prod")
                for k in range(32):
                    nc.vector.tensor_scalar_mul(
                        prod[:, k * W : (k + 1) * W],
                        rep4[:, off : off + W],
                        scal[:, 32 * s + k : 32 * s + k + 1],
                    )
                expt = epool.tile([128, 32 * Wmax], bf16, tag="expt")
                for q in range(4):
                    nc.scalar.activation(
                        expt[:, q * 8 * W : (q + 1) * 8 * W],
                        prod[:, q * 8 * W : (q + 1) * 8 * W],
                        Exp,
                    )
                E_lo = psE.tile([64, Wmax], mybir.dt.float32, tag="Elo")
                E_hi = psE.tile([64, Wmax], mybir.dt.float32, tag="Ehi")
                for h in range(4):
                    E_t = E_lo if h < 2 else E_hi
                    rb = 32 * (h % 2)
                    for i in range(8):
                        k = 8 * h + i
                        nc.tensor.matmul(
                            E_t[rb : rb + 32, 0:W],
                            lhsT=onesbd[:, 32 * i : 32 * (i + 1)],
                            rhs=expt[:, k * W : (k + 1) * W],
                            start=(i == 0),
                            stop=(i == 7),
                        )
                nc.tensor.matmul(
                    Ssum[:, s : s + 1],
                    lhsT=lhsa[:, 128 * s : 128 * (s + 1)],
                    rhs=ysum[:, s : s + 1],
                    start=True,
                    stop=True,
                )
                nc.scalar.activation(
                    logE[0:64, off : off + W], E_lo[:, 0:W], Ln
                )
                nc.scalar.activation(
                    logE[64:128, off : off + W], E_hi[:, 0:W], Ln
                )
                nc.vector.tensor_reduce(
                    Lred[:, s : s + 1],
                    logE[:, off : off + W],
                    axis=mybir.AxisListType.X,
                    op=mybir.AluOpType.add,
                )
                nc.vector.tensor_reduce(
                    Lown[:, s : s + 1],
                    logE[:, off : off + min(128, W)],
                    axis=mybir.AxisListType.X,
                    op=mybir.AluOpType.add,
                )

            mult = mybir.AluOpType.mult
            add = mybir.AluOpType.add
            acc = rpool.tile([128, 1], f32, tag="acc")
            if os.environ.get("USE_TTR", "0") == "1":
                t1 = rpool.tile([128, ns], f32, tag="t1")
                t2 = rpool.tile([128, ns], f32, tag="t2")
                t3 = rpool.tile([128, ns], f32, tag="t3")
                a1 = rpool.tile([128, 1], f32, tag="a1")
                a2 = rpool.tile([128, 1], f32, tag="a2")
                nc.vector.tensor_tensor_reduce(
                    t1[:], Ssum[:], wS[:], 1.0, kw[:, 0:1], mult, add, a1[:, 0:1]
                )
                nc.vector.tensor_tensor_reduce(
                    t2[:], Lred[:], wA[:], 1.0, a1[:, 0:1], mult, add, a2[:, 0:1]
                )
                nc.vector.tensor_tensor_reduce(
                    t3[:], Lown[:], wB[:], 1.0, a2[:, 0:1], mult, add, acc[:, 0:1]
                )
            else:
                # baseline-proven ops only: per-column STT accumulation
                t1 = rpool.tile([128, ns], f32, tag="t1")
                t2 = rpool.tile([128, ns], f32, tag="t2")
                nc.vector.tensor_copy(acc[:], kw[:])
                nc.vector.tensor_mul(t1[:], Ssum[:], wS[:])
                nc.vector.tensor_mul(t2[:], Lred[:], wA[:])
                nc.vector.tensor_add(t1[:], t1[:], t2[:])
                nc.vector.tensor_mul(t2[:], Lown[:], wB[:])
                nc.vector.tensor_add(t1[:], t1[:], t2[:])
                t3 = rpool.tile([128, 1], f32, tag="t3s")
                nc.vector.tensor_reduce(
                    t3[:], t1[:], axis=mybir.AxisListType.X, op=add
                )
                nc.vector.tensor_add(acc[:], acc[:], t3[:])
            nc.gpsimd.dma_start(out_d[:], acc[:])

    nc.compile()
    return nc


def _make_onesbd():
    onesbd = np.zeros((128, 8 * 32), ml_dtypes.bfloat16)
    for r in range(8):
        for a in range(4):
            onesbd[32 * a : 32 * (a + 1), 32 * r + 4 * r + a] = 1.0
    return onesbd


def _host_inputs(ps, widths, per_core, n):
    """Per-core input arrays for the compiled program."""
    ns = len(widths)
    TW = sum(widths)
    offs = [sum(widths[:s]) for s in range(ns)]
    onesbd = _make_onesbd()
    in_maps = []
    for c in range(N_CORES):
        rep4 = np.zeros((128, TW), np.float32)
        scal = np.zeros((128, ns * 32), np.float32)
        lhsa = np.zeros((32, ns * 128), np.float32)
        ysum = np.zeros((32, ns), np.float64)
        ws = np.zeros((128, ns), np.float32)
        wa = np.zeros((128, ns), np.float32)
        wb = np.zeros((128, ns), np.float32)
        kw = np.zeros((128, 1), np.float64)
        for s, slot in enumerate(per_core[c]):
            if slot is None:
                continue
            W = widths[s]
            off = offs[s]
            pos0, lo, hi, P = slot["pos0"], slot["lo"], slot["hi"], slot["P"]
            sc0, sl = slot["sfx"]
            # window real columns
            ywin = np.zeros((32, W), np.float32)
            yw = np.zeros(32, np.float64)
            nreal = sl
            ndo = 0
            if slot["own"] is not None:
                L1 = slot["own"][1]
                ocols = ps[pos0 + lo : pos0 + hi]  # [L1, 32]
                ywin[:, 0:L1] = ocols.T
                ywin[:, 128 : 128 + sl] = ps[sc0 : sc0 + sl].T
                yw = ocols.astype(np.float64).sum(axis=0) + 2.0 * ps[
                    sc0 : sc0 + sl
                ].astype(np.float64).sum(axis=0)
                nreal = L1 + sl
                ndo = 128 - L1
            else:
                ywin[:, 0:sl] = ps[sc0 : sc0 + sl].T
                yw = 2.0 * ps[sc0 : sc0 + sl].astype(np.float64).sum(axis=0)
            rep4[:, off : off + W] = np.tile(ywin, (4, 1))
            ablk = np.zeros((32, 128), np.float32)
            ablk[:, lo:hi] = ps[pos0 + lo : pos0 + hi].T
            lhsa[:, 128 * s : 128 * (s + 1)] = ablk
            scal[:, 32 * s : 32 * (s + 1)] = ablk.T.reshape(32, 128).T
            ysum[:, s] = yw
            wcol = -1.0 / (n * float(P) * float(P))
            ws[lo:hi, s] = wcol
            wa[:, s] = -2.0 * D * ws[:, s]
            if slot["own"] is not None:
                wb[:, s] = D * ws[:, s]
            ndf = W - nreal
            kw[:, 0] -= LN_D * (
                wa[:, s].astype(np.float64) * ndf
                + wb[:, s].astype(np.float64) * ndo
            )
        in_maps.append(
            {
                "rep4": rep4.astype(ml_dtypes.bfloat16),
                "scal": scal,
                "lhsa": lhsa,
                "ysum": ysum.astype(np.float32),
                "ws": ws,
                "wa": wa,
                "wb": wb,
                "kw": kw.astype(np.float32),
                "onesbd": onesbd,
            }
        )
    return in_maps


def _prep(points, sensitive_attribute, t):
    points = np.asarray(points, dtype=np.float32)
    sa = np.asarray(sensitive_attribute).astype(np.int64)
    n, d = points.shape
    assert d == D
    scale = 1.0 / math.sqrt(float(np.asarray(t)))
    order = np.argsort(sa, kind="stable")
    ps = (points[order] * np.float32(scale)).astype(np.float32)
    widths, per_core = _plan(sa[order])
    return ps, widths, per_core, n


def simulate(points, sensitive_attribute, t):
    """Numpy emulation of the device program (for validation)."""
    ps, widths, per_core, n = _prep(points, sensitive_attribute, t)
    in_maps = _host_inputs(ps, widths, per_core, n)
    ns = len(widths)
    offs = [sum(widths[:s]) for s in range(ns)]
    total = 0.0
    for c in range(N_CORES):
        m = in_maps[c]
        rep4 = m["rep4"].astype(np.float32)
        acc = m["kw"][:, 0].astype(np.float64).copy()
        for s in range(ns):
            W = widths[s]
            off = offs[s]
            scal = m["scal"][:, 32 * s : 32 * (s + 1)]  # [128, 32]
            # prod[p, k, j] = rep4[p, off+j] * scal[p, k]
            prod = (
                rep4[:, off : off + W][:, None, :] * scal[:, :, None]
            ).astype(ml_dtypes.bfloat16)
            ex = np.exp(prod.astype(np.float32)).astype(ml_dtypes.bfloat16)
            ex = ex.astype(np.float32)
            # E[a_hat=4k+a, j] = sum_d ex[32a+d, k, j]
            E = np.zeros((128, W), np.float32)
            for k in range(32):
                for a in range(4):
                    E[4 * k + a] = ex[32 * a : 32 * (a + 1), k, :].sum(axis=0)
            lnE = np.log(E)
            Lred = lnE.sum(axis=1)
            Lown = lnE[:, : min(128, W)].sum(axis=1)
            ablk = m["lhsa"][:, 128 * s : 128 * (s + 1)]
            Ssum = ablk.T @ m["ysum"][:, s]
            acc += (
                m["ws"][:, s].astype(np.float64) * Ssum
                + m["wa"][:, s].astype(np.float64) * Lred
                + m["wb"][:, s].astype(np.float64) * Lown
            )
        total += acc.sum()
    return np.float32(total)


def kernel(points, sensitive_attribute, t):
    _install_ntff_hook()
    _install_act_table_patch()

    ps, widths, per_core, n = _prep(points, sensitive_attribute, t)
    in_maps = _host_inputs(ps, widths, per_core, n)

    nc = _build_program(widths)
    trace = bool(int(os.environ.get("KERNEL_TRACE", "0")))
    res = run_bass_kernel_spmd(nc, in_maps, list(range(N_CORES)), trace=trace)
    last_run_info["exec_time_ns"] = res.exec_time_ns
    last_run_info["mean_exec_time_ns"] = res.mean_exec_time_ns
    last_run_info["W"] = widths
    last_run_info["ntiles"] = len(widths)
    last_run_info["instructions"] = (
        res.instructions_and_trace[0] if res.instructions_and_trace else None
    )

    total = 0.0
    for c in range(N_CORES):
        total += float(res.results[c]["out"].astype(np.float64).sum())
    return np.float32(total)
